# revision 11
# baseline (speedup 1.0000x reference)
"""GAT attention head (gnn_message_passing) on 8 trn2 NeuronCores.

Math (per batch b):
    F = seq @ W^T                        [N, 64]
    f1 = F @ a1^T + a1_b                 [N, 1]
    f2 = F @ a2^T + a2_b                 [N, 1]
    logits[i,j] = lrelu(f1[i] + f2[j]) + bias_mat[i,j]
    out = softmax_j(logits) @ F + bias   [N, 64]

Sharding: rows i are split across 8 cores (1024 rows each, both batches).
Each core redundantly computes the full F (cheap) and streams its
[2, 1024, 8192] shard of bias_mat.

Device layout trick: scores are computed TRANSPOSED ([j on partitions,
i on free]) so the softmax matmul P @ F contracts over the partition
dim with no on-chip transposes.  The host passes bias_mat shards
pre-transposed ([2, 8192 j, 1024 i]) -- a pure layout change; the
device still reads all 537 MB.

Per (b, j-group) pipeline:
    S[j,i] = f1[i] + f2[j]; S = max(S, 0.01*S)        (DVE/GPSIMD/ACT)
    S += bias_mat^T tile                              (DMA accum_op=add)
    E = exp(S)                                        (ACT, in place)
    OT[o,i] += V'[j,o]^T @ E[j,i]                     (PE, PSUM accum)
where V' = [F | ones] so OT row 64 is the softmax denominator.
"""

import sys
import numpy as np
from contextlib import ExitStack

sys.path.insert(0, "/opt/trn_rl_repo")

import concourse.bass as bass
import concourse.tile as tile
from concourse import mybir, bacc, masks
from concourse.bass_utils import run_bass_kernel_spmd

FP = mybir.dt.float32
B, N, IN, OUT = 2, 8192, 128, 64
NCORES = 8
LOCAL = N // NCORES          # 1024 rows per core per batch
NCH = N // 128               # 64 j-chunks per batch
JG = 8                       # j-chunks per DMA group
NGRP = NCH // JG             # 8 groups per batch
NEG = 0.01                   # leaky relu slope
ACC_SPLIT = 4                # accum DMAs per group (descriptor length cap)

# which engine computes the f1+f2 outer-sum for sub-slice g of each group:
# 'g' = GPSIMD, 'd' = DVE.  The lrelu (scalar_tensor_tensor max) always
# runs on DVE (gpsimd rejects 2-tensor ops), exp always on ACT.
SLICE_ENG = ['g', 'g', 'g', 'g', 'g', 'g', 'g', 'g']

_CACHED_NC = None


def build_nc():
    nc = bacc.Bacc("TRN2", target_bir_lowering=False, debug=False,
                   num_devices=NCORES)

    # per-core inputs (host prepares layouts; all pure layout transforms)
    # biasT[b, jg, p, g*LOCAL + i] = bias_mat[b, row, jg*1024 + g*128 + p]
    biasT = nc.dram_tensor("biasT", [B, NGRP, 128, JG * LOCAL], FP,
                           kind="ExternalInput").ap()
    seqT = nc.dram_tensor("seqT", [B, IN, N], FP, kind="ExternalInput").ap()
    seqlT = nc.dram_tensor("seqlT", [B, IN, LOCAL], FP, kind="ExternalInput").ap()
    Wn = nc.dram_tensor("Wn", [OUT, IN], FP, kind="ExternalInput").ap()
    WT = nc.dram_tensor("WT", [IN, OUT], FP, kind="ExternalInput").ap()
    a1T = nc.dram_tensor("a1T", [OUT, 1], FP, kind="ExternalInput").ap()
    a2T = nc.dram_tensor("a2T", [OUT, 1], FP, kind="ExternalInput").ap()
    a1b = nc.dram_tensor("a1b", [1, 1], FP, kind="ExternalInput").ap()
    a2b = nc.dram_tensor("a2b", [1, 1], FP, kind="ExternalInput").ap()
    brow = nc.dram_tensor("brow", [1, OUT], FP, kind="ExternalInput").ap()
    out = nc.dram_tensor("out", [B, LOCAL, OUT], FP, kind="ExternalOutput").ap()

    with tile.TileContext(nc) as tc, ExitStack() as ctx:
        # ---- persistent pools -------------------------------------------
        const_pool = ctx.enter_context(tc.tile_pool(name="const", bufs=1))
        feat_pool = ctx.enter_context(tc.tile_pool(name="feat", bufs=1))
        spool = ctx.enter_context(tc.tile_pool(name="scores", bufs=3))
        tpool = ctx.enter_context(tc.tile_pool(name="tmp", bufs=3))
        opool = ctx.enter_context(tc.tile_pool(name="outs", bufs=2))
        setup_ctx = ctx.enter_context(ExitStack())
        ps_one = setup_ctx.enter_context(
            tc.tile_pool(name="ps_one", bufs=1, space="PSUM"))
        ps_small = setup_ctx.enter_context(
            tc.tile_pool(name="ps_small", bufs=2, space="PSUM"))

        # ---- P0: constants ----------------------------------------------
        ident = const_pool.tile([128, 128], FP)
        masks.make_identity(nc, ident[:])
        ones_row = const_pool.tile([1, 128], FP)
        nc.vector.memset(ones_row[:], 1.0)

        Wsb = const_pool.tile([OUT, IN], FP)
        nc.sync.dma_start(Wsb[:], Wn[:])
        rhs66 = const_pool.tile([IN, 66], FP)
        nc.sync.dma_start(rhs66[:, 0:64], WT[:])
        a1sb = const_pool.tile([OUT, 1], FP)
        nc.sync.dma_start(a1sb[:], a1T[:])
        a2sb = const_pool.tile([OUT, 1], FP)
        nc.sync.dma_start(a2sb[:], a2T[:])
        a1bsb = const_pool.tile([1, 1], FP)
        nc.sync.dma_start(a1bsb[:], a1b[:])
        a2bsb = const_pool.tile([1, 1], FP)
        nc.sync.dma_start(a2bsb[:], a2b[:])
        brsb = const_pool.tile([1, OUT], FP)
        nc.sync.dma_start(brsb[:], brow[:])

        # w1 = W^T @ a1 (column IN-vector), w2 likewise -> rhs66 cols 64, 65
        wps = ps_one.tile([IN, 2], FP)
        nc.tensor.matmul(wps[:, 0:1], Wsb[:], a1sb[:], start=True, stop=True)
        nc.tensor.matmul(wps[:, 1:2], Wsb[:], a2sb[:], start=True, stop=True)
        nc.scalar.copy(rhs66[:, 64:66], wps[:])

        # broadcasts across partitions (matmul with ones stationary)
        bps = ps_one.tile([128, OUT + 2], FP)
        nc.tensor.matmul(bps[:, 0:1], ones_row[:], a1bsb[:], start=True, stop=True)
        nc.tensor.matmul(bps[:, 1:2], ones_row[:], a2bsb[:], start=True, stop=True)
        nc.tensor.matmul(bps[:, 2:2 + OUT], ones_row[:], brsb[:], start=True, stop=True)
        a1b_bc = const_pool.tile([128, 1], FP)
        a2b_bc = const_pool.tile([128, 1], FP)
        bias_bc = const_pool.tile([128, OUT], FP)
        nc.scalar.copy(a1b_bc[:], bps[:, 0:1])
        nc.scalar.copy(a2b_bc[:], bps[:, 1:2])
        nc.scalar.copy(bias_bc[:], bps[:, 2:2 + OUT])

        # ---- P1: features -----------------------------------------------
        # Vp: per j-chunk [128, 65] = [F chunk | ones]; both batches.
        Vp = feat_pool.tile([128, B * NCH * 65], FP)
        Vp3 = Vp[:].rearrange("p (n e) -> p n e", e=65)
        nc.vector.memset(Vp3[:, :, 64:65], 1.0)
        f2T = feat_pool.tile([128, B * NCH], FP)
        F1B = feat_pool.tile([128, B * LOCAL], FP)

        for b in range(B):
            for jc in range(NCH):
                nt = b * NCH + jc
                sqt = tpool.tile([128, 128], FP, tag="sqt")
                nc.sync.dma_start(sqt[:], seqT[b][:, jc * 128:(jc + 1) * 128])
                fc = ps_small.tile([128, 66], FP, tag="fc")
                nc.tensor.matmul(fc[:], sqt[:], rhs66[:], start=True, stop=True)
                nc.scalar.copy(Vp3[:, nt, 0:64], fc[:, 0:64])
                nc.vector.tensor_scalar(f2T[:, nt:nt + 1], fc[:, 65:66],
                                        a2b_bc[:], None, mybir.AluOpType.add)

        for b in range(B):
            for il in range(LOCAL // 128):
                slt = tpool.tile([128, 128], FP, tag="sqt")
                nc.sync.dma_start(slt[:], seqlT[b][:, il * 128:(il + 1) * 128])
                flc = ps_small.tile([128, 66], FP, tag="fc")
                nc.tensor.matmul(flc[:], slt[:], rhs66[:], start=True, stop=True)
                f1c = tpool.tile([128, 1], FP, tag="f1c")
                nc.vector.tensor_scalar(f1c[:], flc[:, 64:65], a1b_bc[:], None,
                                        mybir.AluOpType.add)
                f1ct = ps_small.tile([1, 128], FP, tag="f1ct")
                nc.tensor.transpose(f1ct[:], f1c[:], ident[:])
                f1cs = tpool.tile([1, 128], FP, tag="f1cs")
                nc.scalar.copy(f1cs[:], f1ct[:])
                f1bp = ps_small.tile([128, 128], FP, tag="f1bp")
                nc.tensor.matmul(f1bp[:], ones_row[:], f1cs[:], start=True, stop=True)
                nc.scalar.copy(F1B[:, b * LOCAL + il * 128: b * LOCAL + (il + 1) * 128],
                               f1bp[:])

        # ---- P2: main loop ----------------------------------------------
        setup_ctx.close()  # release P0/P1 PSUM banks
        ps_ot = ctx.enter_context(tc.tile_pool(name="ps_ot", bufs=1, space="PSUM"))
        ps_tr = ctx.enter_context(tc.tile_pool(name="ps_tr", bufs=2, space="PSUM"))

        for b in range(B):
            OTp = ps_ot.tile([65, LOCAL], FP)
            F1Bb = F1B[:, b * LOCAL:(b + 1) * LOCAL]
            for jg in range(NGRP):
                S = spool.tile([128, JG * LOCAL], FP, tag="S")
                S3 = S[:].rearrange("p (g i) -> p g i", g=JG)
                for g in range(JG):
                    nt = b * NCH + jg * JG + g
                    f2c = f2T[:, nt:nt + 1]
                    e = nc.gpsimd if SLICE_ENG[g] == 'g' else nc.vector
                    e.tensor_scalar(S3[:, g], F1Bb, f2c, None,
                                    mybir.AluOpType.add)
                    nc.vector.scalar_tensor_tensor(S3[:, g], S3[:, g], NEG,
                                                   S3[:, g],
                                                   mybir.AluOpType.mult,
                                                   mybir.AluOpType.max)
                # += bias_mat^T (during the DMA itself).  Split: the CCE
                # accumulate path dies on contiguous runs > ~8KB.
                npc = JG * LOCAL // ACC_SPLIT
                for k in range(ACC_SPLIT):
                    nc.gpsimd.dma_start(S[:, k * npc:(k + 1) * npc],
                                        biasT[b, jg][:, k * npc:(k + 1) * npc],
                                        accum_op=mybir.AluOpType.add)
                for g in range(JG):
                    nc.scalar.activation(S3[:, g], S3[:, g],
                                         mybir.ActivationFunctionType.Exp)
                for g in range(JG):
                    jc = jg * JG + g
                    nt = b * NCH + jc
                    lhsT = Vp3[:, nt, :]
                    for h in range(LOCAL // 512):
                        nc.tensor.matmul(
                            OTp[:, h * 512:(h + 1) * 512], lhsT,
                            S3[:, g, h * 512:(h + 1) * 512],
                            start=(jc == 0), stop=(jc == NCH - 1))

            # ---- epilogue for batch b: divide by rowsum, add bias, store
            ot_sb = opool.tile([65, LOCAL], FP, tag="ot_sb")
            nc.vector.tensor_copy(ot_sb[:], OTp[:])
            for r in range(LOCAL // 128):
                tp = ps_tr.tile([128, 65], FP, tag="tp")
                nc.tensor.transpose(tp[:], ot_sb[:, r * 128:(r + 1) * 128],
                                    ident[:65, :65])
                rcp = tpool.tile([128, 1], FP, tag="rcp")
                nc.vector.reciprocal(rcp[:], tp[:, 64:65])
                osb = opool.tile([128, OUT], FP, tag="osb")
                nc.vector.scalar_tensor_tensor(osb[:], tp[:, 0:64], rcp[:],
                                               bias_bc[:],
                                               mybir.AluOpType.mult,
                                               mybir.AluOpType.add)
                nc.sync.dma_start(out[b, r * 128:(r + 1) * 128, :], osb[:])

    nc.compile()
    return nc


def get_nc():
    global _CACHED_NC
    if _CACHED_NC is None:
        _CACHED_NC = build_nc()
    return _CACHED_NC


def make_in_maps(inputs):
    seq = np.ascontiguousarray(np.asarray(inputs["seq"], dtype=np.float32))
    bias_mat = np.asarray(inputs["bias_mat"], dtype=np.float32)
    W_w = np.ascontiguousarray(np.asarray(inputs["W_w"], dtype=np.float32))
    a1_w = np.asarray(inputs["a1_w"], dtype=np.float32)
    a1_b = np.asarray(inputs["a1_b"], dtype=np.float32)
    a2_w = np.asarray(inputs["a2_w"], dtype=np.float32)
    a2_b = np.asarray(inputs["a2_b"], dtype=np.float32)
    bias = np.asarray(inputs["bias"], dtype=np.float32)

    seqT = np.ascontiguousarray(seq.transpose(0, 2, 1))
    WT = np.ascontiguousarray(W_w.T)
    a1T = np.ascontiguousarray(a1_w.reshape(OUT, 1))
    a2T = np.ascontiguousarray(a2_w.reshape(OUT, 1))
    a1b = a1_b.reshape(1, 1).astype(np.float32)
    a2b = a2_b.reshape(1, 1).astype(np.float32)
    brow = bias.reshape(1, OUT).astype(np.float32)

    in_maps = []
    for c in range(NCORES):
        sl = slice(c * LOCAL, (c + 1) * LOCAL)
        shard = bias_mat[:, sl, :].reshape(B, LOCAL, NGRP, JG, 128)
        biasT_c = np.ascontiguousarray(
            shard.transpose(0, 2, 4, 3, 1)).reshape(B, NGRP, 128, JG * LOCAL)
        seqlT_c = np.ascontiguousarray(seq[:, sl, :].transpose(0, 2, 1))
        in_maps.append({
            "biasT": biasT_c, "seqT": seqT, "seqlT": seqlT_c,
            "Wn": W_w, "WT": WT, "a1T": a1T, "a2T": a2T,
            "a1b": a1b, "a2b": a2b, "brow": brow,
        })
    return in_maps


def kernel(**inputs) -> np.ndarray:
    nc = get_nc()
    in_maps = make_in_maps(inputs)
    res = run_bass_kernel_spmd(nc, in_maps, list(range(NCORES)))
    full = np.empty((B, N, OUT), dtype=np.float32)
    for c in range(NCORES):
        full[:, c * LOCAL:(c + 1) * LOCAL, :] = res.results[c]["out"]
    return full


if __name__ == "__main__":
    rng = np.random.default_rng(0)
    ins = {
        "seq": rng.standard_normal((B, N, IN), dtype=np.float32),
        "bias_mat": np.zeros((B, N, N), dtype=np.float32),
        "W_w": rng.standard_normal((OUT, IN), dtype=np.float32) * 0.05,
        "a1_w": rng.standard_normal((1, OUT), dtype=np.float32) * 0.05,
        "a1_b": rng.standard_normal((1,), dtype=np.float32) * 0.05,
        "a2_w": rng.standard_normal((1, OUT), dtype=np.float32) * 0.05,
        "a2_b": rng.standard_normal((1,), dtype=np.float32) * 0.05,
        "bias": np.zeros((OUT,), dtype=np.float32),
    }
    out = kernel(**ins)
    print("out", out.shape, out.dtype, float(np.abs(out).mean()))


# revision 16
# speedup vs baseline: 50.7261x; 50.7261x over previous
"""GAT attention head (gnn_message_passing) on 8 trn2 NeuronCores.

Math (per batch b):
    F = seq @ W^T                        [N, 64]
    f1 = F @ a1^T + a1_b                 [N, 1]
    f2 = F @ a2^T + a2_b                 [N, 1]
    logits[i,j] = lrelu(f1[i] + f2[j]) + bias_mat[i,j]
    out = softmax_j(logits) @ F + bias   [N, 64]

Sharding: rows i are split across 8 cores (1024 rows each, both batches).
Each core redundantly computes the full F (cheap) and streams its
[2, 1024, 8192] shard of bias_mat.

Device layout trick: scores are computed TRANSPOSED ([j on partitions,
i on free]) so the softmax matmul P @ F contracts over the partition
dim with no on-chip transposes.  The host passes bias_mat shards
pre-transposed ([2, 8192 j, 1024 i]) -- a pure layout change; the
device still reads all 537 MB.

Per (b, j-group) pipeline:
    S[j,i] = f1[i] + f2[j]; S = max(S, 0.01*S)        (DVE/GPSIMD/ACT)
    S += bias_mat^T tile                              (DMA accum_op=add)
    E = exp(S)                                        (ACT, in place)
    OT[o,i] += V'[j,o]^T @ E[j,i]                     (PE, PSUM accum)
where V' = [F | ones] so OT row 64 is the softmax denominator.
"""

import sys
import numpy as np
from contextlib import ExitStack

sys.path.insert(0, "/opt/trn_rl_repo")

import concourse.bass as bass
import concourse.tile as tile
from concourse import mybir, bacc, masks
from concourse.bass_utils import run_bass_kernel_spmd

FP = mybir.dt.float32
B, N, IN, OUT = 2, 8192, 128, 64
NCORES = 8
LOCAL = N // NCORES          # 1024 rows per core per batch
NCH = N // 128               # 64 j-chunks per batch
JG = 8                       # j-chunks per DMA group
NGRP = NCH // JG             # 8 groups per batch
NEG = 0.01                   # leaky relu slope
ACC_SPLIT = 4                # accum DMAs per group (descriptor length cap)

# which engine computes the f1+f2 outer-sum for sub-slice g of each group:
# 'g' = GPSIMD, 'd' = DVE.  The lrelu (scalar_tensor_tensor max) always
# runs on DVE (gpsimd rejects 2-tensor ops), exp always on ACT.
SLICE_ENG = ['g', 'g', 'g', 'g', 'g', 'g', 'g', 'g']

_CACHED_NC = None


def build_nc(p2_repeat=1):
    nc = bacc.Bacc("TRN2", target_bir_lowering=False, debug=False,
                   num_devices=NCORES)

    # per-core inputs (host prepares layouts; all pure layout transforms)
    # biasT[b, jg, p, g*LOCAL + i] = bias_mat[b, row, jg*1024 + g*128 + p]
    biasT = nc.dram_tensor("biasT", [B, NGRP, 128, JG * LOCAL], FP,
                           kind="ExternalInput").ap()
    seqT = nc.dram_tensor("seqT", [B, IN, N], FP, kind="ExternalInput").ap()
    seqlT = nc.dram_tensor("seqlT", [B, IN, LOCAL], FP, kind="ExternalInput").ap()
    Wn = nc.dram_tensor("Wn", [OUT, IN], FP, kind="ExternalInput").ap()
    WT = nc.dram_tensor("WT", [IN, OUT], FP, kind="ExternalInput").ap()
    a1T = nc.dram_tensor("a1T", [OUT, 1], FP, kind="ExternalInput").ap()
    a2T = nc.dram_tensor("a2T", [OUT, 1], FP, kind="ExternalInput").ap()
    a1b = nc.dram_tensor("a1b", [1, 1], FP, kind="ExternalInput").ap()
    a2b = nc.dram_tensor("a2b", [1, 1], FP, kind="ExternalInput").ap()
    brow = nc.dram_tensor("brow", [1, OUT], FP, kind="ExternalInput").ap()
    out = nc.dram_tensor("out", [B, LOCAL, OUT], FP, kind="ExternalOutput").ap()

    with tile.TileContext(nc) as tc, ExitStack() as ctx:
        # ---- persistent pools -------------------------------------------
        const_pool = ctx.enter_context(tc.tile_pool(name="const", bufs=1))
        feat_pool = ctx.enter_context(tc.tile_pool(name="feat", bufs=1))
        spool = ctx.enter_context(tc.tile_pool(name="scores", bufs=3))
        tpool = ctx.enter_context(tc.tile_pool(name="tmp", bufs=3))
        opool = ctx.enter_context(tc.tile_pool(name="outs", bufs=2))
        setup_ctx = ctx.enter_context(ExitStack())
        ps_one = setup_ctx.enter_context(
            tc.tile_pool(name="ps_one", bufs=1, space="PSUM"))
        ps_small = setup_ctx.enter_context(
            tc.tile_pool(name="ps_small", bufs=2, space="PSUM"))

        # ---- P0: constants ----------------------------------------------
        ident = const_pool.tile([128, 128], FP)
        masks.make_identity(nc, ident[:])
        ones_row = const_pool.tile([1, 128], FP)
        nc.vector.memset(ones_row[:], 1.0)

        Wsb = const_pool.tile([OUT, IN], FP)
        nc.sync.dma_start(Wsb[:], Wn[:])
        rhs66 = const_pool.tile([IN, 66], FP)
        nc.sync.dma_start(rhs66[:, 0:64], WT[:])
        a1sb = const_pool.tile([OUT, 1], FP)
        nc.sync.dma_start(a1sb[:], a1T[:])
        a2sb = const_pool.tile([OUT, 1], FP)
        nc.sync.dma_start(a2sb[:], a2T[:])
        a1bsb = const_pool.tile([1, 1], FP)
        nc.sync.dma_start(a1bsb[:], a1b[:])
        a2bsb = const_pool.tile([1, 1], FP)
        nc.sync.dma_start(a2bsb[:], a2b[:])
        brsb = const_pool.tile([1, OUT], FP)
        nc.sync.dma_start(brsb[:], brow[:])

        # w1 = W^T @ a1 (column IN-vector), w2 likewise -> rhs66 cols 64, 65
        wps = ps_one.tile([IN, 2], FP)
        nc.tensor.matmul(wps[:, 0:1], Wsb[:], a1sb[:], start=True, stop=True)
        nc.tensor.matmul(wps[:, 1:2], Wsb[:], a2sb[:], start=True, stop=True)
        nc.scalar.copy(rhs66[:, 64:66], wps[:])

        # broadcasts across partitions (matmul with ones stationary)
        bps = ps_one.tile([128, OUT + 2], FP)
        nc.tensor.matmul(bps[:, 0:1], ones_row[:], a1bsb[:], start=True, stop=True)
        nc.tensor.matmul(bps[:, 1:2], ones_row[:], a2bsb[:], start=True, stop=True)
        nc.tensor.matmul(bps[:, 2:2 + OUT], ones_row[:], brsb[:], start=True, stop=True)
        a1b_bc = const_pool.tile([128, 1], FP)
        a2b_bc = const_pool.tile([128, 1], FP)
        bias_bc = const_pool.tile([128, OUT], FP)
        nc.scalar.copy(a1b_bc[:], bps[:, 0:1])
        nc.scalar.copy(a2b_bc[:], bps[:, 1:2])
        nc.scalar.copy(bias_bc[:], bps[:, 2:2 + OUT])

        # ---- P1: features -----------------------------------------------
        # Vp: per j-chunk [128, 65] = [F chunk | ones]; both batches.
        Vp = feat_pool.tile([128, B * NCH * 65], FP)
        Vp3 = Vp[:].rearrange("p (n e) -> p n e", e=65)
        nc.vector.memset(Vp3[:, :, 64:65], 1.0)
        f2T = feat_pool.tile([128, B * NCH], FP)
        F1B = feat_pool.tile([128, B * LOCAL], FP)

        for b in range(B):
            for jc in range(NCH):
                nt = b * NCH + jc
                sqt = tpool.tile([128, 128], FP, tag="sqt")
                nc.sync.dma_start(sqt[:], seqT[b][:, jc * 128:(jc + 1) * 128])
                fc = ps_small.tile([128, 66], FP, tag="fc")
                nc.tensor.matmul(fc[:], sqt[:], rhs66[:], start=True, stop=True)
                nc.scalar.copy(Vp3[:, nt, 0:64], fc[:, 0:64])
                nc.vector.tensor_scalar(f2T[:, nt:nt + 1], fc[:, 65:66],
                                        a2b_bc[:], None, mybir.AluOpType.add)

        for b in range(B):
            for il in range(LOCAL // 128):
                slt = tpool.tile([128, 128], FP, tag="sqt")
                nc.sync.dma_start(slt[:], seqlT[b][:, il * 128:(il + 1) * 128])
                flc = ps_small.tile([128, 66], FP, tag="fc")
                nc.tensor.matmul(flc[:], slt[:], rhs66[:], start=True, stop=True)
                f1c = tpool.tile([128, 1], FP, tag="f1c")
                nc.vector.tensor_scalar(f1c[:], flc[:, 64:65], a1b_bc[:], None,
                                        mybir.AluOpType.add)
                f1ct = ps_small.tile([1, 128], FP, tag="f1ct")
                nc.tensor.transpose(f1ct[:], f1c[:], ident[:])
                f1cs = tpool.tile([1, 128], FP, tag="f1cs")
                nc.scalar.copy(f1cs[:], f1ct[:])
                f1bp = ps_small.tile([128, 128], FP, tag="f1bp")
                nc.tensor.matmul(f1bp[:], ones_row[:], f1cs[:], start=True, stop=True)
                nc.scalar.copy(F1B[:, b * LOCAL + il * 128: b * LOCAL + (il + 1) * 128],
                               f1bp[:])

        # ---- P2: main loop ----------------------------------------------
        setup_ctx.close()  # release P0/P1 PSUM banks
        ps_ot = ctx.enter_context(tc.tile_pool(name="ps_ot", bufs=2, space="PSUM"))
        ps_tr = ctx.enter_context(tc.tile_pool(name="ps_tr", bufs=2, space="PSUM"))

        loop_ctx = ExitStack()
        if p2_repeat > 1:
            loop_ctx.enter_context(tc.For_i(0, p2_repeat, 1))

        OTps = {}

        def epilogue(b):
            # divide by rowsum, add bias, store
            OTp = OTps[b]
            ot_sb = opool.tile([65, LOCAL], FP, tag="ot_sb")
            nc.vector.tensor_copy(ot_sb[:], OTp[:])
            for r in range(LOCAL // 128):
                tp = ps_tr.tile([128, 65], FP, tag="tp")
                nc.tensor.transpose(tp[:], ot_sb[:, r * 128:(r + 1) * 128],
                                    ident[:65, :65])
                rcp = tpool.tile([128, 1], FP, tag="rcp")
                nc.vector.reciprocal(rcp[:], tp[:, 64:65])
                osb = opool.tile([128, OUT], FP, tag="osb")
                nc.vector.scalar_tensor_tensor(osb[:], tp[:, 0:64], rcp[:],
                                               bias_bc[:],
                                               mybir.AluOpType.mult,
                                               mybir.AluOpType.add)
                nc.sync.dma_start(out[b, r * 128:(r + 1) * 128, :], osb[:])

        for b in range(B):
            OTp = ps_ot.tile([65, LOCAL], FP)
            OTps[b] = OTp
            F1Bb = F1B[:, b * LOCAL:(b + 1) * LOCAL]
            for jg in range(NGRP):
                S = spool.tile([128, JG * LOCAL], FP, tag="S")
                S3 = S[:].rearrange("p (g i) -> p g i", g=JG)
                for g in range(JG):
                    nt = b * NCH + jg * JG + g
                    f2c = f2T[:, nt:nt + 1]
                    e = nc.gpsimd if SLICE_ENG[g] == 'g' else nc.vector
                    e.tensor_scalar(S3[:, g], F1Bb, f2c, None,
                                    mybir.AluOpType.add)
                    nc.vector.scalar_tensor_tensor(S3[:, g], S3[:, g], NEG,
                                                   S3[:, g],
                                                   mybir.AluOpType.mult,
                                                   mybir.AluOpType.max)
                # += bias_mat^T (during the DMA itself).  Split: the CCE
                # accumulate path dies on contiguous runs > ~8KB.
                npc = JG * LOCAL // ACC_SPLIT
                for k in range(ACC_SPLIT):
                    nc.gpsimd.dma_start(S[:, k * npc:(k + 1) * npc],
                                        biasT[b, jg][:, k * npc:(k + 1) * npc],
                                        accum_op=mybir.AluOpType.add)
                for g in range(JG):
                    nc.scalar.activation(S3[:, g], S3[:, g],
                                         mybir.ActivationFunctionType.Exp)
                for g in range(JG):
                    jc = jg * JG + g
                    nt = b * NCH + jc
                    lhsT = Vp3[:, nt, :]
                    for h in range(LOCAL // 512):
                        nc.tensor.matmul(
                            OTp[:, h * 512:(h + 1) * 512], lhsT,
                            S3[:, g, h * 512:(h + 1) * 512],
                            start=(jc == 0), stop=(jc == NCH - 1))

            if p2_repeat == 1:
                epilogue(b)

        loop_ctx.close()
        if p2_repeat > 1:
            for b in range(B):
                epilogue(b)

    nc.compile()
    return nc


def get_nc():
    global _CACHED_NC
    if _CACHED_NC is None:
        _CACHED_NC = build_nc()
    return _CACHED_NC


def make_in_maps(inputs):
    seq = np.ascontiguousarray(np.asarray(inputs["seq"], dtype=np.float32))
    bias_mat = np.asarray(inputs["bias_mat"], dtype=np.float32)
    W_w = np.ascontiguousarray(np.asarray(inputs["W_w"], dtype=np.float32))
    a1_w = np.asarray(inputs["a1_w"], dtype=np.float32)
    a1_b = np.asarray(inputs["a1_b"], dtype=np.float32)
    a2_w = np.asarray(inputs["a2_w"], dtype=np.float32)
    a2_b = np.asarray(inputs["a2_b"], dtype=np.float32)
    bias = np.asarray(inputs["bias"], dtype=np.float32)

    seqT = np.ascontiguousarray(seq.transpose(0, 2, 1))
    WT = np.ascontiguousarray(W_w.T)
    a1T = np.ascontiguousarray(a1_w.reshape(OUT, 1))
    a2T = np.ascontiguousarray(a2_w.reshape(OUT, 1))
    a1b = a1_b.reshape(1, 1).astype(np.float32)
    a2b = a2_b.reshape(1, 1).astype(np.float32)
    brow = bias.reshape(1, OUT).astype(np.float32)

    in_maps = []
    for c in range(NCORES):
        sl = slice(c * LOCAL, (c + 1) * LOCAL)
        shard = bias_mat[:, sl, :].reshape(B, LOCAL, NGRP, JG, 128)
        biasT_c = np.ascontiguousarray(
            shard.transpose(0, 2, 4, 3, 1)).reshape(B, NGRP, 128, JG * LOCAL)
        seqlT_c = np.ascontiguousarray(seq[:, sl, :].transpose(0, 2, 1))
        in_maps.append({
            "biasT": biasT_c, "seqT": seqT, "seqlT": seqlT_c,
            "Wn": W_w, "WT": WT, "a1T": a1T, "a2T": a2T,
            "a1b": a1b, "a2b": a2b, "brow": brow,
        })
    return in_maps


def kernel(**inputs) -> np.ndarray:
    nc = get_nc()
    in_maps = make_in_maps(inputs)
    res = run_bass_kernel_spmd(nc, in_maps, list(range(NCORES)))
    full = np.empty((B, N, OUT), dtype=np.float32)
    for c in range(NCORES):
        full[:, c * LOCAL:(c + 1) * LOCAL, :] = res.results[c]["out"]
    return full


if __name__ == "__main__":
    rng = np.random.default_rng(0)
    ins = {
        "seq": rng.standard_normal((B, N, IN), dtype=np.float32),
        "bias_mat": np.zeros((B, N, N), dtype=np.float32),
        "W_w": rng.standard_normal((OUT, IN), dtype=np.float32) * 0.05,
        "a1_w": rng.standard_normal((1, OUT), dtype=np.float32) * 0.05,
        "a1_b": rng.standard_normal((1,), dtype=np.float32) * 0.05,
        "a2_w": rng.standard_normal((1, OUT), dtype=np.float32) * 0.05,
        "a2_b": rng.standard_normal((1,), dtype=np.float32) * 0.05,
        "bias": np.zeros((OUT,), dtype=np.float32),
    }
    out = kernel(**ins)
    print("out", out.shape, out.dtype, float(np.abs(out).mean()))


# revision 18
# speedup vs baseline: 86.1631x; 1.6986x over previous
"""GAT attention head (gnn_message_passing) on 8 trn2 NeuronCores.

Math (per batch b):
    F = seq @ W^T                        [N, 64]
    f1 = F @ a1^T + a1_b                 [N, 1]
    f2 = F @ a2^T + a2_b                 [N, 1]
    logits[i,j] = lrelu(f1[i] + f2[j]) + bias_mat[i,j]
    out = softmax_j(logits) @ F + bias   [N, 64]

Sharding: rows i are split across 8 cores (1024 rows each, both batches).
Each core redundantly computes the full F (cheap) and streams its
[2, 1024, 8192] shard of bias_mat.

Device layout trick: scores are computed TRANSPOSED ([j on partitions,
i on free]) so the softmax matmul P @ F contracts over the partition
dim with no on-chip transposes.  The host passes bias_mat shards
pre-transposed ([2, 8192 j, 1024 i]) -- a pure layout change; the
device still reads all 537 MB.

Per (b, j-group) pipeline:
    S[j,i] = f1[i] + f2[j]; S = max(S, 0.01*S)        (DVE/GPSIMD/ACT)
    S += bias_mat^T tile                              (DMA accum_op=add)
    E = exp(S)                                        (ACT, in place)
    OT[o,i] += V'[j,o]^T @ E[j,i]                     (PE, PSUM accum)
where V' = [F | ones] so OT row 64 is the softmax denominator.
"""

import sys
import numpy as np
from contextlib import ExitStack

sys.path.insert(0, "/opt/trn_rl_repo")

import concourse.bass as bass
import concourse.tile as tile
from concourse import mybir, bacc, masks
from concourse.bass_utils import run_bass_kernel_spmd
import concourse.dve_ops as _D
from concourse.dve_spec import Spec as _Spec, Src0 as _Src0, Src1 as _Src1, \
    C0 as _C0, C1 as _C1, maxx as _maxx, lower as _lower
from concourse.dve_uop import DveOpSpec as _DveOpSpec


def _register_fused_op():
    """out = lrelu(in1 + s0) + in0, slope s1 -- one DVE pass for the whole
    score stage (outer-sum + leaky-relu + bias add)."""
    name = "GAT_SCORE_FUSED_ANT"
    for op in _D.OPS:
        if op.name == name:
            return op
    s = _Src1 + _C0
    spec = _Spec(body=_maxx(s, s * _C1) + _Src0,
                 reference=lambda in0, in1, s0, s1, imm2:
                     np.maximum(in1 + s0, (in1 + s0) * s1) + in0)
    opcode = _D._CUSTOM_DVE_ROW_BASE + len(_D.OPS)
    shas = {}
    for ver in ("v3", "v4"):
        dspec = _DveOpSpec(name=name, opcode=opcode,
                           uops=_lower(spec, ver=ver), rd1_en=True)
        shas[ver] = dspec.sha(ver)
    op = _D.DveOp(name, spec, subdim=False, uops_sha=shas)
    _D.OPS.append(op)
    _D._SUB_OPCODE_FOR_NAME[name] = opcode
    _D.CUSTOM_DVE_SPECS[name] = spec
    return op


FUSED_OP = _register_fused_op()

FP = mybir.dt.float32
B, N, IN, OUT = 2, 8192, 128, 64
NCORES = 8
LOCAL = N // NCORES          # 1024 rows per core per batch
NCH = N // 128               # 64 j-chunks per batch
JG = 8                       # j-chunks per DMA group
NGRP = NCH // JG             # 8 groups per batch
NEG = 0.01                   # leaky relu slope
ACC_SPLIT = 4                # accum DMAs per group (descriptor length cap)

# which engine computes the f1+f2 outer-sum for sub-slice g of each group:
# 'g' = GPSIMD, 'd' = DVE.  The lrelu (scalar_tensor_tensor max) always
# runs on DVE (gpsimd rejects 2-tensor ops), exp always on ACT.
SLICE_ENG = ['g', 'g', 'g', 'g', 'g', 'g', 'g', 'g']

_CACHED_NC = None


def build_nc(p2_repeat=1, slice_eng=None, use_accum=True, do_elem=True,
             do_exp=True, acc_split=None, sbufs=3, use_fused=True):
    slice_eng = slice_eng or SLICE_ENG
    acc_split = acc_split or ACC_SPLIT
    nc = bacc.Bacc("TRN2", target_bir_lowering=False, debug=False,
                   num_devices=NCORES)

    # per-core inputs (host prepares layouts; all pure layout transforms)
    # biasT[b, jg, p, g*LOCAL + i] = bias_mat[b, row, jg*1024 + g*128 + p]
    biasT = nc.dram_tensor("biasT", [B, NGRP, 128, JG * LOCAL], FP,
                           kind="ExternalInput").ap()
    seqT = nc.dram_tensor("seqT", [B, IN, N], FP, kind="ExternalInput").ap()
    seqlT = nc.dram_tensor("seqlT", [B, IN, LOCAL], FP, kind="ExternalInput").ap()
    Wn = nc.dram_tensor("Wn", [OUT, IN], FP, kind="ExternalInput").ap()
    WT = nc.dram_tensor("WT", [IN, OUT], FP, kind="ExternalInput").ap()
    a1T = nc.dram_tensor("a1T", [OUT, 1], FP, kind="ExternalInput").ap()
    a2T = nc.dram_tensor("a2T", [OUT, 1], FP, kind="ExternalInput").ap()
    a1b = nc.dram_tensor("a1b", [1, 1], FP, kind="ExternalInput").ap()
    a2b = nc.dram_tensor("a2b", [1, 1], FP, kind="ExternalInput").ap()
    brow = nc.dram_tensor("brow", [1, OUT], FP, kind="ExternalInput").ap()
    out = nc.dram_tensor("out", [B, LOCAL, OUT], FP, kind="ExternalOutput").ap()

    with tile.TileContext(nc) as tc, ExitStack() as ctx:
        # ---- persistent pools -------------------------------------------
        const_pool = ctx.enter_context(tc.tile_pool(name="const", bufs=1))
        feat_pool = ctx.enter_context(tc.tile_pool(name="feat", bufs=1))
        spool = ctx.enter_context(tc.tile_pool(name="scores", bufs=sbufs))
        tpool = ctx.enter_context(tc.tile_pool(name="tmp", bufs=3))
        opool = ctx.enter_context(tc.tile_pool(name="outs", bufs=2))
        setup_ctx = ctx.enter_context(ExitStack())
        ps_one = setup_ctx.enter_context(
            tc.tile_pool(name="ps_one", bufs=1, space="PSUM"))
        ps_small = setup_ctx.enter_context(
            tc.tile_pool(name="ps_small", bufs=2, space="PSUM"))

        # ---- P0: constants ----------------------------------------------
        ident = const_pool.tile([128, 128], FP)
        masks.make_identity(nc, ident[:])
        ones_row = const_pool.tile([1, 128], FP)
        nc.vector.memset(ones_row[:], 1.0)

        Wsb = const_pool.tile([OUT, IN], FP)
        nc.sync.dma_start(Wsb[:], Wn[:])
        rhs66 = const_pool.tile([IN, 66], FP)
        nc.sync.dma_start(rhs66[:, 0:64], WT[:])
        a1sb = const_pool.tile([OUT, 1], FP)
        nc.sync.dma_start(a1sb[:], a1T[:])
        a2sb = const_pool.tile([OUT, 1], FP)
        nc.sync.dma_start(a2sb[:], a2T[:])
        a1bsb = const_pool.tile([1, 1], FP)
        nc.sync.dma_start(a1bsb[:], a1b[:])
        a2bsb = const_pool.tile([1, 1], FP)
        nc.sync.dma_start(a2bsb[:], a2b[:])
        brsb = const_pool.tile([1, OUT], FP)
        nc.sync.dma_start(brsb[:], brow[:])

        # w1 = W^T @ a1 (column IN-vector), w2 likewise -> rhs66 cols 64, 65
        wps = ps_one.tile([IN, 2], FP)
        nc.tensor.matmul(wps[:, 0:1], Wsb[:], a1sb[:], start=True, stop=True)
        nc.tensor.matmul(wps[:, 1:2], Wsb[:], a2sb[:], start=True, stop=True)
        nc.scalar.copy(rhs66[:, 64:66], wps[:])

        # broadcasts across partitions (matmul with ones stationary)
        bps = ps_one.tile([128, OUT + 2], FP)
        nc.tensor.matmul(bps[:, 0:1], ones_row[:], a1bsb[:], start=True, stop=True)
        nc.tensor.matmul(bps[:, 1:2], ones_row[:], a2bsb[:], start=True, stop=True)
        nc.tensor.matmul(bps[:, 2:2 + OUT], ones_row[:], brsb[:], start=True, stop=True)
        a1b_bc = const_pool.tile([128, 1], FP)
        a2b_bc = const_pool.tile([128, 1], FP)
        bias_bc = const_pool.tile([128, OUT], FP)
        nc.scalar.copy(a1b_bc[:], bps[:, 0:1])
        nc.scalar.copy(a2b_bc[:], bps[:, 1:2])
        nc.scalar.copy(bias_bc[:], bps[:, 2:2 + OUT])

        # ---- P1: features -----------------------------------------------
        # Vp: per j-chunk [128, 65] = [F chunk | ones]; both batches.
        Vp = feat_pool.tile([128, B * NCH * 65], FP)
        Vp3 = Vp[:].rearrange("p (n e) -> p n e", e=65)
        nc.vector.memset(Vp3[:, :, 64:65], 1.0)
        f2T = feat_pool.tile([128, B * NCH], FP)
        F1B = feat_pool.tile([128, B * LOCAL], FP)

        for b in range(B):
            for jc in range(NCH):
                nt = b * NCH + jc
                sqt = tpool.tile([128, 128], FP, tag="sqt")
                nc.sync.dma_start(sqt[:], seqT[b][:, jc * 128:(jc + 1) * 128])
                fc = ps_small.tile([128, 66], FP, tag="fc")
                nc.tensor.matmul(fc[:], sqt[:], rhs66[:], start=True, stop=True)
                nc.scalar.copy(Vp3[:, nt, 0:64], fc[:, 0:64])
                nc.vector.tensor_scalar(f2T[:, nt:nt + 1], fc[:, 65:66],
                                        a2b_bc[:], None, mybir.AluOpType.add)

        for b in range(B):
            for il in range(LOCAL // 128):
                slt = tpool.tile([128, 128], FP, tag="sqt")
                nc.sync.dma_start(slt[:], seqlT[b][:, il * 128:(il + 1) * 128])
                flc = ps_small.tile([128, 66], FP, tag="fc")
                nc.tensor.matmul(flc[:], slt[:], rhs66[:], start=True, stop=True)
                f1c = tpool.tile([128, 1], FP, tag="f1c")
                nc.vector.tensor_scalar(f1c[:], flc[:, 64:65], a1b_bc[:], None,
                                        mybir.AluOpType.add)
                f1ct = ps_small.tile([1, 128], FP, tag="f1ct")
                nc.tensor.transpose(f1ct[:], f1c[:], ident[:])
                f1cs = tpool.tile([1, 128], FP, tag="f1cs")
                nc.scalar.copy(f1cs[:], f1ct[:])
                f1bp = ps_small.tile([128, 128], FP, tag="f1bp")
                nc.tensor.matmul(f1bp[:], ones_row[:], f1cs[:], start=True, stop=True)
                nc.scalar.copy(F1B[:, b * LOCAL + il * 128: b * LOCAL + (il + 1) * 128],
                               f1bp[:])

        # ---- P2: main loop ----------------------------------------------
        setup_ctx.close()  # release P0/P1 PSUM banks
        ps_ot = ctx.enter_context(tc.tile_pool(name="ps_ot", bufs=2, space="PSUM"))
        ps_tr = ctx.enter_context(tc.tile_pool(name="ps_tr", bufs=2, space="PSUM"))

        loop_ctx = ExitStack()
        if p2_repeat > 1:
            loop_ctx.enter_context(tc.For_i(0, p2_repeat, 1))

        OTps = {}

        def epilogue(b):
            # divide by rowsum, add bias, store
            OTp = OTps[b]
            ot_sb = opool.tile([65, LOCAL], FP, tag="ot_sb")
            nc.vector.tensor_copy(ot_sb[:], OTp[:])
            for r in range(LOCAL // 128):
                tp = ps_tr.tile([128, 65], FP, tag="tp")
                nc.tensor.transpose(tp[:], ot_sb[:, r * 128:(r + 1) * 128],
                                    ident[:65, :65])
                rcp = tpool.tile([128, 1], FP, tag="rcp")
                nc.vector.reciprocal(rcp[:], tp[:, 64:65])
                osb = opool.tile([128, OUT], FP, tag="osb")
                nc.vector.scalar_tensor_tensor(osb[:], tp[:, 0:64], rcp[:],
                                               bias_bc[:],
                                               mybir.AluOpType.mult,
                                               mybir.AluOpType.add)
                nc.sync.dma_start(out[b, r * 128:(r + 1) * 128, :], osb[:])

        for b in range(B):
            OTp = ps_ot.tile([65, LOCAL], FP)
            OTps[b] = OTp
            F1Bb = F1B[:, b * LOCAL:(b + 1) * LOCAL]
            for jg in range(NGRP):
                S = spool.tile([128, JG * LOCAL], FP, tag="S")
                S3 = S[:].rearrange("p (g i) -> p g i", g=JG)
                if use_fused:
                    # prefetchable plain load, then one fused DVE pass/slice
                    nc.sync.dma_start(S[:], biasT[b, jg])
                    if do_elem:
                        for g in range(JG):
                            nt = b * NCH + jg * JG + g
                            nc.vector._custom_dve(
                                FUSED_OP, out=S3[:, g], in0=S3[:, g],
                                in1=F1Bb, s0=f2T[:, nt:nt + 1], s1=NEG)
                    if do_exp:
                        nc.scalar.activation(S[:], S[:],
                                             mybir.ActivationFunctionType.Exp)
                else:
                    if do_elem:
                        for g in range(JG):
                            nt = b * NCH + jg * JG + g
                            f2c = f2T[:, nt:nt + 1]
                            e = nc.gpsimd if slice_eng[g] == 'g' else nc.vector
                            e.tensor_scalar(S3[:, g], F1Bb, f2c, None,
                                            mybir.AluOpType.add)
                            nc.vector.scalar_tensor_tensor(
                                S3[:, g], S3[:, g], NEG, S3[:, g],
                                mybir.AluOpType.mult, mybir.AluOpType.max)
                    npc = JG * LOCAL // acc_split
                    for k in range(acc_split):
                        if use_accum:
                            nc.gpsimd.dma_start(
                                S[:, k * npc:(k + 1) * npc],
                                biasT[b, jg][:, k * npc:(k + 1) * npc],
                                accum_op=mybir.AluOpType.add)
                        else:
                            nc.sync.dma_start(
                                S[:, k * npc:(k + 1) * npc],
                                biasT[b, jg][:, k * npc:(k + 1) * npc])
                    if do_exp:
                        for g in range(JG):
                            nc.scalar.activation(S3[:, g], S3[:, g],
                                                 mybir.ActivationFunctionType.Exp)
                for g in range(JG):
                    jc = jg * JG + g
                    nt = b * NCH + jc
                    lhsT = Vp3[:, nt, :]
                    for h in range(LOCAL // 512):
                        nc.tensor.matmul(
                            OTp[:, h * 512:(h + 1) * 512], lhsT,
                            S3[:, g, h * 512:(h + 1) * 512],
                            start=(jc == 0), stop=(jc == NCH - 1))

            if p2_repeat == 1:
                epilogue(b)

        loop_ctx.close()
        if p2_repeat > 1:
            for b in range(B):
                epilogue(b)

    nc.compile()
    return nc


def get_nc():
    global _CACHED_NC
    if _CACHED_NC is None:
        _CACHED_NC = build_nc()
    return _CACHED_NC


def make_in_maps(inputs):
    seq = np.ascontiguousarray(np.asarray(inputs["seq"], dtype=np.float32))
    bias_mat = np.asarray(inputs["bias_mat"], dtype=np.float32)
    W_w = np.ascontiguousarray(np.asarray(inputs["W_w"], dtype=np.float32))
    a1_w = np.asarray(inputs["a1_w"], dtype=np.float32)
    a1_b = np.asarray(inputs["a1_b"], dtype=np.float32)
    a2_w = np.asarray(inputs["a2_w"], dtype=np.float32)
    a2_b = np.asarray(inputs["a2_b"], dtype=np.float32)
    bias = np.asarray(inputs["bias"], dtype=np.float32)

    seqT = np.ascontiguousarray(seq.transpose(0, 2, 1))
    WT = np.ascontiguousarray(W_w.T)
    a1T = np.ascontiguousarray(a1_w.reshape(OUT, 1))
    a2T = np.ascontiguousarray(a2_w.reshape(OUT, 1))
    a1b = a1_b.reshape(1, 1).astype(np.float32)
    a2b = a2_b.reshape(1, 1).astype(np.float32)
    brow = bias.reshape(1, OUT).astype(np.float32)

    in_maps = []
    for c in range(NCORES):
        sl = slice(c * LOCAL, (c + 1) * LOCAL)
        shard = bias_mat[:, sl, :].reshape(B, LOCAL, NGRP, JG, 128)
        biasT_c = np.ascontiguousarray(
            shard.transpose(0, 2, 4, 3, 1)).reshape(B, NGRP, 128, JG * LOCAL)
        seqlT_c = np.ascontiguousarray(seq[:, sl, :].transpose(0, 2, 1))
        in_maps.append({
            "biasT": biasT_c, "seqT": seqT, "seqlT": seqlT_c,
            "Wn": W_w, "WT": WT, "a1T": a1T, "a2T": a2T,
            "a1b": a1b, "a2b": a2b, "brow": brow,
        })
    return in_maps


def kernel(**inputs) -> np.ndarray:
    nc = get_nc()
    in_maps = make_in_maps(inputs)
    res = run_bass_kernel_spmd(nc, in_maps, list(range(NCORES)))
    full = np.empty((B, N, OUT), dtype=np.float32)
    for c in range(NCORES):
        full[:, c * LOCAL:(c + 1) * LOCAL, :] = res.results[c]["out"]
    return full


if __name__ == "__main__":
    rng = np.random.default_rng(0)
    ins = {
        "seq": rng.standard_normal((B, N, IN), dtype=np.float32),
        "bias_mat": np.zeros((B, N, N), dtype=np.float32),
        "W_w": rng.standard_normal((OUT, IN), dtype=np.float32) * 0.05,
        "a1_w": rng.standard_normal((1, OUT), dtype=np.float32) * 0.05,
        "a1_b": rng.standard_normal((1,), dtype=np.float32) * 0.05,
        "a2_w": rng.standard_normal((1, OUT), dtype=np.float32) * 0.05,
        "a2_b": rng.standard_normal((1,), dtype=np.float32) * 0.05,
        "bias": np.zeros((OUT,), dtype=np.float32),
    }
    out = kernel(**ins)
    print("out", out.shape, out.dtype, float(np.abs(out).mean()))
